# revision 3
# baseline (speedup 1.0000x reference)
"""LukeSelfAttention Trainium2 kernel.

Batch data-parallel across 8 NeuronCores (B=8 -> 1 batch per core).

Per-core plan (all matmuls bf16 operands, fp32 PSUM accumulation):
  - Host pre-transposes: xT = concat(word, ent).T  [1024, 640] bf16,
    WT[m] = W_m.T [in, out] bf16 for the 6 projection matrices.
  - K/Q projections in transposed layout: KT/QT[o, s] = W @ xT,
    bias fused into the PSUM->SBUF copy as a per-partition scalar.
  - V projection in natural layout V[s, o] (lhsT = xT tile, rhs = WvT),
    bias added via a rank-1 (ones x bvT) matmul starting the accumulation
    group; copied into a stride-65 interleaved layout Vaug[s, h*65+d] with
    a ones column at d=64 per head.
  - Scores computed transposed, sT[k, q] = Kh^T.T @ Qh^T, per (head, k-tile):
    word-query block N=512, entity-query block N=128.  attention mask is a
    per-partition (per-key) scalar -> exp(s*0.125 + mask) is ONE ScalarE
    activation from PSUM into bf16 SBUF (probsT, unnormalized).
  - Context: ctx[q, 65] += probsT_tile.T @ Vaug[k, h*65:+65]; column 64
    accumulates the softmax denominator.  Normalize with
    vector.reciprocal + tensor_scalar mult (fp32) into the ctx SBUF tile.
  - Output DMA: ctx rows 0:512 -> word out, 512:640 -> entity out.
"""

import numpy as np
import ml_dtypes

import concourse.bass as bass
import concourse.bacc as bacc
import concourse.mybir as mybir
import concourse.tile as tile
from concourse.bass_utils import run_bass_kernel_spmd

F32 = mybir.dt.float32
F32R = mybir.dt.float32r
BF16 = mybir.dt.bfloat16

HIDDEN = 1024
HEADS = 16
DH = 64          # head dim
SW = 512         # word seq
SE = 128         # entity seq
S = SW + SE      # 640
NCORES = 8
NI = HIDDEN // 128   # 8 input-feature tiles
NO = HIDDEN // 128   # 8 output-feature tiles
NK = S // 128        # 5 key tiles
NQ = S // 128        # 5 query tiles

# weight order in the packed [6, 1024, 1024] tensor
M_K, M_V, M_QW, M_W2E, M_E2W, M_E2E = 0, 1, 2, 3, 4, 5

_CACHED_NC = None


def _emit(tc):
    nc = tc.nc
    xT_d = nc.dram_tensor("xT", [HIDDEN, S], BF16, kind="ExternalInput").ap()
    wt_d = nc.dram_tensor("wt", [6, HIDDEN, HIDDEN], BF16, kind="ExternalInput").ap()
    bias_d = nc.dram_tensor("biases", [128, 48], F32, kind="ExternalInput").ap()
    mask_d = nc.dram_tensor("mask", [128, NK], F32, kind="ExternalInput").ap()
    bvT_d = nc.dram_tensor("bvT", [1, HIDDEN], BF16, kind="ExternalInput").ap()
    out_w = nc.dram_tensor("out_w", [SW, HIDDEN], F32, kind="ExternalOutput").ap()
    out_e = nc.dram_tensor("out_e", [SE, HIDDEN], F32, kind="ExternalOutput").ap()

    with (
        tc.tile_pool(name="persist", bufs=1) as pp,
        tc.tile_pool(name="wstream", bufs=16) as wp,
        tc.tile_pool(name="expp", bufs=12) as ep,
        tc.tile_pool(name="smallp", bufs=6) as sp,
        tc.tile_pool(name="psbig", bufs=2, space="PSUM") as ps_big,
        tc.tile_pool(name="psproj", bufs=2, space="PSUM") as ps_proj,
        tc.tile_pool(name="psctx", bufs=2, space="PSUM") as ps_ctx,
    ):
        # ---- persistent SBUF tensors ----
        bias_sb = pp.tile([128, 48], F32, name="bias_sb", tag="bias_sb")
        mask_sb = pp.tile([128, NK], F32, name="mask_sb", tag="mask_sb")
        bvT_sb = pp.tile([1, HIDDEN], BF16, name="bvT_sb", tag="bvT_sb")
        ones1 = pp.tile([1, 128], BF16, name="ones1", tag="ones1")
        xT = [pp.tile([128, S], BF16, name=f"xT{i}", tag=f"xT{i}") for i in range(NI)]
        KT = [pp.tile([128, S], BF16, name=f"KT{t}", tag=f"KT{t}") for t in range(NO)]
        qww = [pp.tile([128, SW], BF16, name=f"qww{t}", tag=f"qww{t}") for t in range(NO)]
        qwe = [pp.tile([128, SW], BF16, name=f"qwe{t}", tag=f"qwe{t}") for t in range(NO)]
        qew = [pp.tile([128, SE], BF16, name=f"qew{t}", tag=f"qew{t}") for t in range(NO)]
        qee = [pp.tile([128, SE], BF16, name=f"qee{t}", tag=f"qee{t}") for t in range(NO)]
        vaug = [pp.tile([128, HEADS * (DH + 1)], BF16, name=f"vaug{s}", tag=f"vaug{s}")
                for s in range(NK)]
        ctx = [pp.tile([128, HIDDEN], F32, name=f"ctx{q}", tag=f"ctx{q}") for q in range(NQ)]
        # Q-projection weights stay resident (reused by nothing, but loaded early
        # so their DMAs overlap the K/V projection compute)
        wq = {m: [pp.tile([128, HIDDEN], BF16, name=f"wq{m}_{i}", tag=f"wq{m}_{i}")
                  for i in range(NI)] for m in (M_QW, M_W2E, M_E2W, M_E2E)}

        # ---- input DMAs ----
        nc.sync.dma_start(out=bias_sb[:], in_=bias_d[:])
        nc.sync.dma_start(out=mask_sb[:], in_=mask_d[:])
        nc.sync.dma_start(out=bvT_sb[:], in_=bvT_d[:])
        nc.vector.memset(ones1[:], 1.0)
        for i in range(NI):
            nc.sync.dma_start(out=xT[i][:], in_=xT_d[i * 128:(i + 1) * 128, :])

        wk = [wp.tile([128, HIDDEN], BF16, name=f"wk{i}", tag="w") for i in range(NI)]
        for i in range(NI):
            nc.sync.dma_start(out=wk[i][:], in_=wt_d[M_K, i * 128:(i + 1) * 128, :])
        wv = [wp.tile([128, HIDDEN], BF16, name=f"wv{i}", tag="w") for i in range(NI)]
        for i in range(NI):
            nc.sync.dma_start(out=wv[i][:], in_=wt_d[M_V, i * 128:(i + 1) * 128, :])
        for m in (M_QW, M_W2E, M_E2W, M_E2E):
            for i in range(NI):
                nc.sync.dma_start(out=wq[m][i][:], in_=wt_d[m, i * 128:(i + 1) * 128, :])

        # ---- K projection: KT[o, s] ----
        for t in range(NO):
            ps = ps_big.tile([128, S], F32, name="kps", tag="sps")
            for i in range(NI):
                nc.tensor.matmul(ps[:, 0:SW], wk[i][:, t * 128:(t + 1) * 128],
                                 xT[i][:, 0:SW], start=(i == 0), stop=(i == NI - 1))
            for i in range(NI):
                nc.tensor.matmul(ps[:, SW:S], wk[i][:, t * 128:(t + 1) * 128],
                                 xT[i][:, SW:S], start=(i == 0), stop=(i == NI - 1))
            nc.vector.tensor_scalar_add(KT[t][:], ps[:], bias_sb[:, M_K * 8 + t: M_K * 8 + t + 1])

        # ---- V projection: V[s, o] with bias via rank-1 matmul; write into
        # stride-65 interleaved vaug, then set the ones columns ----
        for st in range(NK):
            for oc in range(2):
                ps = ps_proj.tile([128, 512], F32, name="vps", tag="pps")
                nc.tensor.matmul(ps[:], ones1[:],
                                 bvT_sb[:, oc * 512:(oc + 1) * 512],
                                 start=True, stop=False)
                for i in range(NI):
                    nc.tensor.matmul(ps[:], xT[i][:, st * 128:(st + 1) * 128],
                                     wv[i][:, oc * 512:(oc + 1) * 512],
                                     start=False, stop=(i == NI - 1))
                dst = vaug[st].rearrange("p (h c) -> p h c", c=DH + 1)[:, oc * 8:(oc + 1) * 8, 0:DH]
                src = ps.rearrange("p (h c) -> p h c", c=DH)
                nc.vector.tensor_copy(dst, src)
            onescol = vaug[st].rearrange("p (h c) -> p h c", c=DH + 1)[:, :, DH:DH + 1]
            nc.vector.memset(onescol, 1.0)

        # ---- per head-pair: Q projections for o-tile p, then attention ----
        for p in range(NO):
            # word-query projections (w2w, w2e): QT[o, s_w]
            for m, dst in ((M_QW, qww), (M_W2E, qwe)):
                ps = ps_proj.tile([128, 512], F32, name="qps", tag="pps")
                for i in range(NI):
                    nc.tensor.matmul(ps[:], wq[m][i][:, p * 128:(p + 1) * 128],
                                     xT[i][:, 0:SW], start=(i == 0), stop=(i == NI - 1))
                nc.vector.tensor_scalar_add(dst[p][:], ps[:], bias_sb[:, m * 8 + p: m * 8 + p + 1])
            # entity-query projections (e2w, e2e): QT[o, s_e]
            for m, dst in ((M_E2W, qew), (M_E2E, qee)):
                ps = ps_proj.tile([128, SE], F32, name="qeps", tag="pps")
                for i in range(NI):
                    nc.tensor.matmul(ps[:], wq[m][i][:, p * 128:(p + 1) * 128],
                                     xT[i][:, SW:S], start=(i == 0), stop=(i == NI - 1))
                nc.vector.tensor_scalar_add(dst[p][:], ps[:], bias_sb[:, m * 8 + p: m * 8 + p + 1])

            # attention for heads 2p (partitions 0:64) and 2p+1 (partitions 64:128)
            probs = {0: [], 1: []}
            for kt in range(NK):
                wq_arr = qww if kt < 4 else qwe
                eq_arr = qew if kt < 4 else qee
                st_tiles = {}
                for hh in (0, 1):
                    b0 = hh * DH
                    lhsT = KT[p][b0:b0 + DH, kt * 128:(kt + 1) * 128]
                    ps = ps_big.tile([128, S], F32, name=f"sps{hh}", tag="sps")
                    nc.tensor.matmul(ps[:, 0:SW], lhsT, wq_arr[p][b0:b0 + DH, :],
                                     start=True, stop=True)
                    nc.tensor.matmul(ps[:, SW:S], lhsT, eq_arr[p][b0:b0 + DH, :],
                                     start=True, stop=True)
                    st_tiles[hh] = ps
                for hh in (0, 1):
                    et = ep.tile([128, S], BF16, name=f"expT{hh}", tag="expT")
                    nc.scalar.activation(et[:], st_tiles[hh][:],
                                         mybir.ActivationFunctionType.Exp,
                                         bias=mask_sb[:, kt:kt + 1], scale=0.125)
                    probs[hh].append(et)

            for hh in (0, 1):
                h = 2 * p + hh
                for qt in range(NQ):
                    cps = ps_ctx.tile([128, DH + 1], F32, name="cps", tag="cps")
                    for kt in range(NK):
                        nc.tensor.matmul(cps[:], probs[hh][kt][:, qt * 128:(qt + 1) * 128],
                                         vaug[kt][:, h * (DH + 1):(h + 1) * (DH + 1)],
                                         start=(kt == 0), stop=(kt == NK - 1))
                    recip = sp.tile([128, 1], F32, name="recip", tag="recip")
                    nc.vector.reciprocal(recip[:], cps[:, DH:DH + 1])
                    nc.vector.tensor_scalar_mul(ctx[qt][:, h * DH:(h + 1) * DH],
                                                cps[:, 0:DH], recip[:])

        # ---- output ----
        for qt in range(4):
            nc.sync.dma_start(out=out_w[qt * 128:(qt + 1) * 128, :], in_=ctx[qt][:])
        nc.sync.dma_start(out=out_e[:], in_=ctx[4][:])


def build_nc():
    nc = bacc.Bacc("TRN2", target_bir_lowering=False, debug=False, num_devices=NCORES)
    with tile.TileContext(nc) as tc:
        _emit(tc)
    nc.compile()
    return nc


def _get_nc():
    global _CACHED_NC
    if _CACHED_NC is None:
        _CACHED_NC = build_nc()
    return _CACHED_NC


def make_in_maps(word_hidden_states, entity_hidden_states, attention_mask,
                 Wq, bq, Wk, bk, Wv, bv, Ww2e, bw2e, We2w, be2w, We2e, be2e):
    word = np.asarray(word_hidden_states, np.float32)
    ent = np.asarray(entity_hidden_states, np.float32)
    amask = np.asarray(attention_mask, np.float32).reshape(-1, S)  # [B, 640]
    B = word.shape[0]

    ws = [np.asarray(w, np.float32) for w in (Wk, Wv, Wq, Ww2e, We2w, We2e)]
    bs = [np.asarray(b, np.float32) for b in (bk, bv, bq, bw2e, be2w, be2e)]
    wt = np.stack([np.ascontiguousarray(w.T) for w in ws]).astype(ml_dtypes.bfloat16)
    # biases[p, m*8+t] = b_m[t*128 + p]
    biases = np.ascontiguousarray(
        np.stack(bs).reshape(6, 8, 128).transpose(2, 0, 1).reshape(128, 48))
    bvT = np.ascontiguousarray(np.asarray(bv, np.float32).reshape(1, HIDDEN)).astype(ml_dtypes.bfloat16)

    in_maps = []
    for b in range(B):
        x = np.concatenate([word[b], ent[b]], axis=0)          # [640, 1024]
        xT = np.ascontiguousarray(x.T).astype(ml_dtypes.bfloat16)
        m = np.ascontiguousarray(amask[b].reshape(NK, 128).T)  # [128, 5]
        in_maps.append({"xT": xT, "wt": wt, "biases": biases, "mask": m, "bvT": bvT})
    return in_maps


def run(in_maps, trace=False):
    nc = _get_nc()
    return run_bass_kernel_spmd(nc, in_maps, core_ids=list(range(len(in_maps))),
                                trace=trace)


def kernel(**inputs):
    in_maps = make_in_maps(**inputs)
    res = run(in_maps)
    out_w = np.stack([res.results[b]["out_w"] for b in range(len(in_maps))])
    out_e = np.stack([res.results[b]["out_e"] for b in range(len(in_maps))])
    return out_w, out_e


# revision 4
# speedup vs baseline: 878.7822x; 878.7822x over previous
"""LukeSelfAttention Trainium2 kernel.

Batch data-parallel across 8 NeuronCores (B=8 -> 1 batch per core).

Per-core plan (all matmuls bf16 operands, fp32 PSUM accumulation):
  - Host pre-transposes: xT = concat(word, ent).T  [1024, 640] bf16,
    WT[m] = W_m.T [in, out] bf16 for the 6 projection matrices.
  - K/Q projections in transposed layout: KT/QT[o, s] = W @ xT,
    bias fused into the PSUM->SBUF copy as a per-partition scalar.
  - V projection in natural layout V[s, o] (lhsT = xT tile, rhs = WvT),
    bias added via a rank-1 (ones x bvT) matmul starting the accumulation
    group; copied into a stride-65 interleaved layout Vaug[s, h*65+d] with
    a ones column at d=64 per head.
  - Scores computed transposed, sT[k, q] = Kh^T.T @ Qh^T, per (head, k-tile):
    word-query block N=512, entity-query block N=128.  attention mask is a
    per-partition (per-key) scalar -> exp(s*0.125 + mask) is ONE ScalarE
    activation from PSUM into bf16 SBUF (probsT, unnormalized).
  - Context: ctx[q, 65] += probsT_tile.T @ Vaug[k, h*65:+65]; column 64
    accumulates the softmax denominator.  Normalize with
    vector.reciprocal + tensor_scalar mult (fp32) into the ctx SBUF tile.
  - Output DMA: ctx rows 0:512 -> word out, 512:640 -> entity out.
"""

import numpy as np
import ml_dtypes

import concourse.bass as bass
import concourse.bacc as bacc
import concourse.mybir as mybir
import concourse.tile as tile
from concourse.bass_utils import run_bass_kernel_spmd

F32 = mybir.dt.float32
F32R = mybir.dt.float32r
BF16 = mybir.dt.bfloat16

HIDDEN = 1024
HEADS = 16
DH = 64          # head dim
SW = 512         # word seq
SE = 128         # entity seq
S = SW + SE      # 640
NCORES = 8
NI = HIDDEN // 128   # 8 input-feature tiles
NO = HIDDEN // 128   # 8 output-feature tiles
NK = S // 128        # 5 key tiles
NQ = S // 128        # 5 query tiles

# weight order in the packed [6, 1024, 1024] tensor
M_K, M_V, M_QW, M_W2E, M_E2W, M_E2E = 0, 1, 2, 3, 4, 5

_CACHED_NC = None


def _emit(tc, loop_reps=None):
    nc = tc.nc
    xT_d = nc.dram_tensor("xT", [HIDDEN, S], BF16, kind="ExternalInput").ap()
    wt_d = nc.dram_tensor("wt", [6, HIDDEN, HIDDEN], BF16, kind="ExternalInput").ap()
    bias_d = nc.dram_tensor("biases", [128, 48], F32, kind="ExternalInput").ap()
    mask_d = nc.dram_tensor("mask", [128, NK], F32, kind="ExternalInput").ap()
    bvT_d = nc.dram_tensor("bvT", [1, HIDDEN], BF16, kind="ExternalInput").ap()
    out_w = nc.dram_tensor("out_w", [SW, HIDDEN], F32, kind="ExternalOutput").ap()
    out_e = nc.dram_tensor("out_e", [SE, HIDDEN], F32, kind="ExternalOutput").ap()

    with (
        tc.tile_pool(name="persist", bufs=1) as pp,
        tc.tile_pool(name="wstream", bufs=16) as wp,
        tc.tile_pool(name="expp", bufs=12) as ep,
        tc.tile_pool(name="smallp", bufs=6) as sp,
        tc.tile_pool(name="psbig", bufs=2, space="PSUM") as ps_big,
        tc.tile_pool(name="psproj", bufs=2, space="PSUM") as ps_proj,
        tc.tile_pool(name="psctx", bufs=2, space="PSUM") as ps_ctx,
    ):
        # ---- persistent SBUF tensors ----
        bias_sb = pp.tile([128, 48], F32, name="bias_sb", tag="bias_sb")
        mask_sb = pp.tile([128, NK], F32, name="mask_sb", tag="mask_sb")
        bvT_sb = pp.tile([1, HIDDEN], BF16, name="bvT_sb", tag="bvT_sb")
        ones1 = pp.tile([1, 128], BF16, name="ones1", tag="ones1")
        xT = [pp.tile([128, S], BF16, name=f"xT{i}", tag=f"xT{i}") for i in range(NI)]
        KT = [pp.tile([128, S], BF16, name=f"KT{t}", tag=f"KT{t}") for t in range(NO)]
        qww = [pp.tile([128, SW], BF16, name=f"qww{t}", tag=f"qww{t}") for t in range(NO)]
        qwe = [pp.tile([128, SW], BF16, name=f"qwe{t}", tag=f"qwe{t}") for t in range(NO)]
        qew = [pp.tile([128, SE], BF16, name=f"qew{t}", tag=f"qew{t}") for t in range(NO)]
        qee = [pp.tile([128, SE], BF16, name=f"qee{t}", tag=f"qee{t}") for t in range(NO)]
        vaug = [pp.tile([128, HEADS * (DH + 1)], BF16, name=f"vaug{s}", tag=f"vaug{s}")
                for s in range(NK)]
        ctx = [pp.tile([128, HIDDEN], F32, name=f"ctx{q}", tag=f"ctx{q}") for q in range(NQ)]
        # Q-projection weights stay resident (reused by nothing, but loaded early
        # so their DMAs overlap the K/V projection compute)
        wq = {m: [pp.tile([128, HIDDEN], BF16, name=f"wq{m}_{i}", tag=f"wq{m}_{i}")
                  for i in range(NI)] for m in (M_QW, M_W2E, M_E2W, M_E2E)}

        def body(_iv=None):
            _body(nc, locals_d)

        locals_d = dict(
            bias_sb=bias_sb, mask_sb=mask_sb, bvT_sb=bvT_sb, ones1=ones1,
            xT=xT, KT=KT, qww=qww, qwe=qwe, qew=qew, qee=qee, vaug=vaug,
            ctx=ctx, wq=wq, wp=wp, ep=ep, sp=sp, ps_big=ps_big,
            ps_proj=ps_proj, ps_ctx=ps_ctx,
            xT_d=xT_d, wt_d=wt_d, bias_d=bias_d, mask_d=mask_d, bvT_d=bvT_d,
            out_w=out_w, out_e=out_e)

        if loop_reps is None:
            body()
        else:
            with tc.For_i(0, loop_reps, 1):
                body()


def _body(nc, d):
    bias_sb = d["bias_sb"]; mask_sb = d["mask_sb"]; bvT_sb = d["bvT_sb"]
    ones1 = d["ones1"]; xT = d["xT"]; KT = d["KT"]; qww = d["qww"]
    qwe = d["qwe"]; qew = d["qew"]; qee = d["qee"]; vaug = d["vaug"]
    ctx = d["ctx"]; wq = d["wq"]; wp = d["wp"]; ep = d["ep"]; sp = d["sp"]
    ps_big = d["ps_big"]; ps_proj = d["ps_proj"]; ps_ctx = d["ps_ctx"]
    xT_d = d["xT_d"]; wt_d = d["wt_d"]; bias_d = d["bias_d"]
    mask_d = d["mask_d"]; bvT_d = d["bvT_d"]; out_w = d["out_w"]; out_e = d["out_e"]

    if True:
        # ---- input DMAs ----
        nc.sync.dma_start(out=bias_sb[:], in_=bias_d[:])
        nc.sync.dma_start(out=mask_sb[:], in_=mask_d[:])
        nc.sync.dma_start(out=bvT_sb[:], in_=bvT_d[:])
        nc.vector.memset(ones1[:], 1.0)
        for i in range(NI):
            nc.sync.dma_start(out=xT[i][:], in_=xT_d[i * 128:(i + 1) * 128, :])

        wk = [wp.tile([128, HIDDEN], BF16, name=f"wk{i}", tag="w") for i in range(NI)]
        for i in range(NI):
            nc.sync.dma_start(out=wk[i][:], in_=wt_d[M_K, i * 128:(i + 1) * 128, :])
        wv = [wp.tile([128, HIDDEN], BF16, name=f"wv{i}", tag="w") for i in range(NI)]
        for i in range(NI):
            nc.sync.dma_start(out=wv[i][:], in_=wt_d[M_V, i * 128:(i + 1) * 128, :])
        for m in (M_QW, M_W2E, M_E2W, M_E2E):
            for i in range(NI):
                nc.sync.dma_start(out=wq[m][i][:], in_=wt_d[m, i * 128:(i + 1) * 128, :])

        # ---- K projection: KT[o, s] ----
        for t in range(NO):
            ps = ps_big.tile([128, S], F32, name="kps", tag="sps")
            for i in range(NI):
                nc.tensor.matmul(ps[:, 0:SW], wk[i][:, t * 128:(t + 1) * 128],
                                 xT[i][:, 0:SW], start=(i == 0), stop=(i == NI - 1))
            for i in range(NI):
                nc.tensor.matmul(ps[:, SW:S], wk[i][:, t * 128:(t + 1) * 128],
                                 xT[i][:, SW:S], start=(i == 0), stop=(i == NI - 1))
            nc.vector.tensor_scalar_add(KT[t][:], ps[:], bias_sb[:, M_K * 8 + t: M_K * 8 + t + 1])

        # ---- V projection: V[s, o] with bias via rank-1 matmul; write into
        # stride-65 interleaved vaug, then set the ones columns ----
        for st in range(NK):
            for oc in range(2):
                ps = ps_proj.tile([128, 512], F32, name="vps", tag="pps")
                nc.tensor.matmul(ps[:], ones1[:],
                                 bvT_sb[:, oc * 512:(oc + 1) * 512],
                                 start=True, stop=False)
                for i in range(NI):
                    nc.tensor.matmul(ps[:], xT[i][:, st * 128:(st + 1) * 128],
                                     wv[i][:, oc * 512:(oc + 1) * 512],
                                     start=False, stop=(i == NI - 1))
                dst = vaug[st].rearrange("p (h c) -> p h c", c=DH + 1)[:, oc * 8:(oc + 1) * 8, 0:DH]
                src = ps.rearrange("p (h c) -> p h c", c=DH)
                nc.vector.tensor_copy(dst, src)
            onescol = vaug[st].rearrange("p (h c) -> p h c", c=DH + 1)[:, :, DH:DH + 1]
            nc.vector.memset(onescol, 1.0)

        # ---- per head-pair: Q projections for o-tile p, then attention ----
        for p in range(NO):
            # word-query projections (w2w, w2e): QT[o, s_w]
            for m, dst in ((M_QW, qww), (M_W2E, qwe)):
                ps = ps_proj.tile([128, 512], F32, name="qps", tag="pps")
                for i in range(NI):
                    nc.tensor.matmul(ps[:], wq[m][i][:, p * 128:(p + 1) * 128],
                                     xT[i][:, 0:SW], start=(i == 0), stop=(i == NI - 1))
                nc.vector.tensor_scalar_add(dst[p][:], ps[:], bias_sb[:, m * 8 + p: m * 8 + p + 1])
            # entity-query projections (e2w, e2e): QT[o, s_e]
            for m, dst in ((M_E2W, qew), (M_E2E, qee)):
                ps = ps_proj.tile([128, SE], F32, name="qeps", tag="pps")
                for i in range(NI):
                    nc.tensor.matmul(ps[:], wq[m][i][:, p * 128:(p + 1) * 128],
                                     xT[i][:, SW:S], start=(i == 0), stop=(i == NI - 1))
                nc.vector.tensor_scalar_add(dst[p][:], ps[:], bias_sb[:, m * 8 + p: m * 8 + p + 1])

            # attention for heads 2p (partitions 0:64) and 2p+1 (partitions 64:128)
            probs = {0: [], 1: []}
            for kt in range(NK):
                wq_arr = qww if kt < 4 else qwe
                eq_arr = qew if kt < 4 else qee
                st_tiles = {}
                for hh in (0, 1):
                    b0 = hh * DH
                    lhsT = KT[p][b0:b0 + DH, kt * 128:(kt + 1) * 128]
                    ps = ps_big.tile([128, S], F32, name=f"sps{hh}", tag="sps")
                    nc.tensor.matmul(ps[:, 0:SW], lhsT, wq_arr[p][b0:b0 + DH, :],
                                     start=True, stop=True)
                    nc.tensor.matmul(ps[:, SW:S], lhsT, eq_arr[p][b0:b0 + DH, :],
                                     start=True, stop=True)
                    st_tiles[hh] = ps
                for hh in (0, 1):
                    et = ep.tile([128, S], BF16, name=f"expT{hh}", tag="expT")
                    nc.scalar.activation(et[:], st_tiles[hh][:],
                                         mybir.ActivationFunctionType.Exp,
                                         bias=mask_sb[:, kt:kt + 1], scale=0.125)
                    probs[hh].append(et)

            for hh in (0, 1):
                h = 2 * p + hh
                for qt in range(NQ):
                    cps = ps_ctx.tile([128, DH + 1], F32, name="cps", tag="cps")
                    for kt in range(NK):
                        nc.tensor.matmul(cps[:], probs[hh][kt][:, qt * 128:(qt + 1) * 128],
                                         vaug[kt][:, h * (DH + 1):(h + 1) * (DH + 1)],
                                         start=(kt == 0), stop=(kt == NK - 1))
                    recip = sp.tile([128, 1], F32, name="recip", tag="recip")
                    nc.vector.reciprocal(recip[:], cps[:, DH:DH + 1])
                    nc.vector.tensor_scalar_mul(ctx[qt][:, h * DH:(h + 1) * DH],
                                                cps[:, 0:DH], recip[:])

        # ---- output ----
        for qt in range(4):
            nc.sync.dma_start(out=out_w[qt * 128:(qt + 1) * 128, :], in_=ctx[qt][:])
        nc.sync.dma_start(out=out_e[:], in_=ctx[4][:])


def build_nc(loop_reps=None):
    nc = bacc.Bacc("TRN2", target_bir_lowering=False, debug=False, num_devices=NCORES)
    with tile.TileContext(nc) as tc:
        _emit(tc, loop_reps=loop_reps)
    nc.compile()
    return nc


def _get_nc():
    global _CACHED_NC
    if _CACHED_NC is None:
        _CACHED_NC = build_nc()
    return _CACHED_NC


def make_in_maps(word_hidden_states, entity_hidden_states, attention_mask,
                 Wq, bq, Wk, bk, Wv, bv, Ww2e, bw2e, We2w, be2w, We2e, be2e):
    word = np.asarray(word_hidden_states, np.float32)
    ent = np.asarray(entity_hidden_states, np.float32)
    amask = np.asarray(attention_mask, np.float32).reshape(-1, S)  # [B, 640]
    B = word.shape[0]

    ws = [np.asarray(w, np.float32) for w in (Wk, Wv, Wq, Ww2e, We2w, We2e)]
    bs = [np.asarray(b, np.float32) for b in (bk, bv, bq, bw2e, be2w, be2e)]
    wt = np.stack([np.ascontiguousarray(w.T) for w in ws]).astype(ml_dtypes.bfloat16)
    # biases[p, m*8+t] = b_m[t*128 + p]
    biases = np.ascontiguousarray(
        np.stack(bs).reshape(6, 8, 128).transpose(2, 0, 1).reshape(128, 48))
    bvT = np.ascontiguousarray(np.asarray(bv, np.float32).reshape(1, HIDDEN)).astype(ml_dtypes.bfloat16)

    in_maps = []
    for b in range(B):
        x = np.concatenate([word[b], ent[b]], axis=0)          # [640, 1024]
        xT = np.ascontiguousarray(x.T).astype(ml_dtypes.bfloat16)
        m = np.ascontiguousarray(amask[b].reshape(NK, 128).T)  # [128, 5]
        in_maps.append({"xT": xT, "wt": wt, "biases": biases, "mask": m, "bvT": bvT})
    return in_maps


def run(in_maps, trace=False):
    nc = _get_nc()
    return run_bass_kernel_spmd(nc, in_maps, core_ids=list(range(len(in_maps))),
                                trace=trace)


def kernel(**inputs):
    in_maps = make_in_maps(**inputs)
    res = run(in_maps)
    out_w = np.stack([res.results[b]["out_w"] for b in range(len(in_maps))])
    out_e = np.stack([res.results[b]["out_e"] for b in range(len(in_maps))])
    return out_w, out_e
